# revision 17
# baseline (speedup 1.0000x reference)
"""Trainium2 Bass kernel for nn_Affinity (gnn_message_passing).

M[(a,b),(c,d)] = sum_{j,i} H2[a,j]H2[c,j] H1[b,i]H1[d,i] W[j,i] + diag(Mp).

Structure exploited:
 - Nonzero blocks (a,c) of M: a==c or (a,c) an edge of graph 2 -> "slots".
   626 slots total, balanced 9 bands/core across 8 cores (<=79 slots/core).
 - Within a block, only graph-1-adjacent (b,d) positions (and the diagonal)
   are nonzero; blocks are symmetric in (b,d), so each block is fully
   described by 72 diagonal values + one value per unique adjacent pair
   (275 of them) -> device output is [80 slots, 352] per core.
 - Per-slot weights factor through ZS = Xsum^T SELT (the edge-affinity
   matrix Me is never materialized), and the block values are
   OUTD = ZS^T (vv H1^T), OUTO = ZS^T (vv R) where R merges multi-edges
   of graph 1 into unique pairs. diag(Mp) folds in as an extra PSUM
   accumulation against an identity table.

All index-derived tables (incidence, SELT, R, H1T, OHSS, I72) are
host-built 0/1 matrices; every floating-point op runs on device. Host
assembly only places computed values (and zeros) into the [5184, 5184]
output.
"""
import sys
sys.path.insert(0, '/opt/trn_rl_repo')
import numpy as np

N = 72
E = 288
D = 64
NC = 8


def _split_waits(nc, limit=1):
    """This walrus build rejects instructions with >limit sem waits; move the
    excess onto same-engine NoOps inserted immediately before (same bb order =
    same engine program order, so semantics are preserved)."""
    import concourse.mybir as mybir
    for f in nc.m.functions:
        for bb in f.blocks:
            new_insts = []
            for inst in bb.instructions:
                si = inst.sync_info
                waits = list(si.on_wait) if si and si.on_wait else []
                if len(waits) > limit:
                    extra, keep = waits[:-limit], waits[-limit:]
                    for i in range(0, len(extra), limit):
                        nop = mybir.InstNoOp(
                            name=nc.get_next_instruction_name(),
                            engine=inst.engine, ins=[], outs=[],
                            sync_info=mybir.SyncInfo(
                                on_wait=extra[i:i + limit], on_update=[]),
                        )
                        nc.register_instruction(nop)
                        new_insts.append(nop)
                    si.on_wait = keep
                new_insts.append(inst)
            bb.instructions[:] = new_insts


def _incidence(src, dst):
    H = np.zeros((N, E), np.float32)
    H[src, np.arange(E)] = 1.0
    H[dst, np.arange(E)] = 1.0
    return H


def _neighbors(src, dst):
    nbrs = [set() for _ in range(N)]
    for s, d in zip(src, dst):
        nbrs[int(s)].add(int(d))
        nbrs[int(d)].add(int(s))
    return nbrs


def _plan_assignment(nbrs2, spad):
    """9 bands per core, greedily balancing slot count (1 + deg per band)."""
    deg = [len(x) for x in nbrs2]
    order = sorted(range(N), key=lambda a: -deg[a])
    cores = [[] for _ in range(NC)]
    loads = [0] * NC
    for a in order:
        c = min((c for c in range(NC) if len(cores[c]) < 9),
                key=lambda c: loads[c])
        cores[c].append(a)
        loads[c] += 1 + deg[a]
    assert max(loads) <= spad
    return cores


def _build_nc(SPAD, NUPAD):
    import concourse.bass as bass
    import concourse.mybir as mybir
    import concourse.tile as tile

    F32 = mybir.dt.float32
    BF16 = mybir.dt.bfloat16
    OUTW = 72 + NUPAD
    CW = NUPAD + 72      # combo width [s2r|s2h] / [d2r|d2h]

    nc = bass.Bass()
    # pk72: f1 [s2r|s2h] [d2r|d2h] hs ; p64x: f2t l1t l2t u1sel u2t
    W72 = D + 2 * CW + SPAD
    W64 = 72 + 2 * D + SPAD + 72
    pk72_d = nc.declare_dram_parameter("PK72", [72, W72], BF16, isOutput=False)
    p64x_d = nc.declare_dram_parameter("P64X", [64, W64], BF16, isOutput=False)
    out_d = nc.declare_dram_parameter("OUT", [SPAD, OUTW], F32, isOutput=True)

    with tile.TileContext(nc) as tc:
        with tc.tile_pool(name="cst", bufs=1) as cst, \
             tc.tile_pool(name="ps", bufs=4, space="PSUM") as ps, \
             tc.tile_pool(name="psb", bufs=4, space="PSUM") as psb:

            pk72 = cst.tile([72, W72], BF16)
            p64x = cst.tile([64, W64], BF16)
            nc.scalar.dma_start(out=p64x[:], in_=p64x_d[:])
            nc.sync.dma_start(out=pk72[:], in_=pk72_d[:])

            f1 = pk72[:, 0:D]
            sr = pk72[:, D:D + CW]                   # [s2r|s2h]
            dr = pk72[:, D + CW:D + 2 * CW]          # [d2r|d2h]
            hs = pk72[:, D + 2 * CW:D + 2 * CW + SPAD]
            f2t = p64x[:, 0:72]
            l1t = p64x[:, 72:72 + D]
            l2t = p64x[:, 72 + D:72 + 2 * D]
            u1sel = p64x[:, 72 + 2 * D:72 + 2 * D + SPAD]
            u2t = p64x[:, 72 + 2 * D + SPAD:W64]

            # relu(lamda^T) (DVE, straight from the small pack)
            r1t = cst.tile([D, D], BF16, tag="r1t")
            nc.vector.tensor_relu(out=r1t[:], in_=l1t)
            r2t = cst.tile([D, D], BF16, tag="r2t")
            nc.vector.tensor_relu(out=r2t[:], in_=l2t)

            # --- PE wave 1 (small-pack gated): AFT = F2 relu(L1)^T etc.
            aft_p = ps.tile([72, D], F32, tag="mm")
            nc.tensor.matmul(out=aft_p[:], lhsT=f2t, rhs=r1t[:],
                             start=True, stop=True)
            bft_p = ps.tile([72, D], F32, tag="mm")
            nc.tensor.matmul(out=bft_p[:], lhsT=f2t, rhs=r2t[:],
                             start=True, stop=True)
            zs_p = ps.tile([D, SPAD], F32, tag="mm")
            nc.tensor.matmul(out=zs_p[:], lhsT=f1, rhs=hs, start=True, stop=True)

            aftc = cst.tile([72, D], BF16, tag="aftc")
            nc.scalar.copy(out=aftc[:], in_=aft_p[:])
            bftc = cst.tile([72, D], BF16, tag="bftc")
            nc.vector.tensor_copy(out=bftc[:], in_=bft_p[:])
            zsc = cst.tile([D, SPAD], BF16, tag="zsc")
            nc.vector.tensor_copy(out=zsc[:], in_=zs_p[:])

            # --- PE wave 2: lamda-applied tables straight from the pack.
            vvk_p = psb.tile([D, CW], F32, tag="bb")
            nc.tensor.matmul(out=vvk_p[:], lhsT=aftc[:], rhs=sr,
                             start=True, stop=False)
            nc.tensor.matmul(out=vvk_p[:], lhsT=bftc[:], rhs=dr,
                             start=False, stop=True)

            vvkcB = cst.tile([D, NUPAD], BF16, tag="vvkcB")
            nc.vector.tensor_copy(out=vvkcB[:], in_=vvk_p[:, 0:NUPAD])
            vvkcA = cst.tile([D, 72], BF16, tag="vvkcA")
            nc.scalar.copy(out=vvkcA[:], in_=vvk_p[:, NUPAD:CW])

            # --- PE wave 3: OUT[s, 72:] = ZS^T vvm; OUT[s, :72] += Mp diag.
            bpB = psb.tile([SPAD, NUPAD], F32, tag="bb")
            nc.tensor.matmul(out=bpB[:], lhsT=zsc[:], rhs=vvkcB[:],
                             start=True, stop=True)
            bpA = psb.tile([SPAD, 72], F32, tag="bb")
            nc.tensor.matmul(out=bpA[:], lhsT=zsc[:], rhs=vvkcA[:],
                             start=True, stop=False)
            # diag(Mp) add: bpA[s, b] += sum_d U1[a_s, d] U2[b, d]
            nc.tensor.matmul(out=bpA[:], lhsT=u1sel, rhs=u2t,
                             start=False, stop=True)

            stgB = cst.tile([SPAD, NUPAD], F32)
            nc.scalar.copy(out=stgB[:], in_=bpB[:])
            nc.sync.dma_start(out=out_d[:, 72:OUTW], in_=stgB[:])
            stgA = cst.tile([SPAD, 72], F32)
            nc.vector.tensor_copy(out=stgA[:], in_=bpA[:])
            nc.scalar.dma_start(out=out_d[:, 0:72], in_=stgA[:])

    _split_waits(nc)
    return nc


def _prepare(inputs):
    import ml_dtypes
    ins = {k: np.asarray(v) for k, v in inputs.items()}
    F1 = ins["F1"].astype(np.float32)
    F2 = ins["F2"].astype(np.float32)
    U1 = ins["U1"].astype(np.float32)
    U2 = ins["U2"].astype(np.float32)
    l1 = ins["lamda1"].astype(np.float32)
    l2 = ins["lamda2"].astype(np.float32)
    src1 = ins["src1"].astype(np.int64)
    dst1 = ins["dst1"].astype(np.int64)
    src2 = ins["src2"].astype(np.int64)
    dst2 = ins["dst2"].astype(np.int64)

    H1 = _incidence(src1, dst1)
    H2 = _incidence(src2, dst2)
    S2 = np.zeros((N, E), np.float32)
    S2[src2, np.arange(E)] = 1.0
    D2M = np.zeros((N, E), np.float32)
    D2M[dst2, np.arange(E)] = 1.0

    nbrs2 = _neighbors(src2, dst2)
    # unique unordered adjacent pairs of graph 1 + multi-edge merge R
    pairs = {}
    for i, (s, d) in enumerate(zip(src1, dst1)):
        pairs.setdefault((int(s), int(d)), []).append(i)
    plist = sorted(pairs)
    NU = len(plist)
    NUPAD = (NU + 7) // 8 * 8
    deg2 = [1 + len(x) for x in nbrs2]
    SPAD = max(80, (max(deg2) + sum(deg2) // NC + 7) // 8 * 8)
    cores = _plan_assignment(nbrs2, SPAD)

    R = np.zeros((E, NUPAD), np.float32)
    for u, key in enumerate(plist):
        for i in pairs[key]:
            R[i, u] = 1.0

    bf = ml_dtypes.bfloat16
    # host-precontracted integer tables (exact in bf16)
    S2R = S2 @ R
    D2R = D2M @ R
    S2H = S2 @ H1.T
    D2H = D2M @ H1.T

    CW = NUPAD + 72
    W72 = D + 2 * CW + SPAD
    W64 = 72 + 2 * D + SPAD + 72
    PK72B = np.zeros((72, W72), bf)
    PK72B[:, 0:D] = F1.astype(bf)
    PK72B[:, D:D + NUPAD] = S2R.astype(bf)
    PK72B[:, D + NUPAD:D + CW] = S2H.astype(bf)
    PK72B[:, D + CW:D + CW + NUPAD] = D2R.astype(bf)
    PK72B[:, D + CW + NUPAD:D + 2 * CW] = D2H.astype(bf)

    in_maps = []
    slot_maps = []
    for c in range(NC):
        slots = []
        for a in cores[c]:
            slots.append((a, a))
            for cc in sorted(nbrs2[a]):
                slots.append((a, cc))
        SELT = np.zeros((E, SPAD), np.float32)
        for s_i, (a, cc) in enumerate(slots):
            SELT[:, s_i] = H2[a] * H2[cc]
        pk72 = PK72B.copy()
        pk72[:, D + 2 * CW:D + 2 * CW + SPAD] = (H1 @ SELT).astype(bf)
        # per-core small pack with U1SEL[d, s] = U1[a_s, d] on diag slots
        U1SEL = np.zeros((64, SPAD), np.float32)
        for s_i, (a, cc) in enumerate(slots):
            if a == cc:
                U1SEL[:, s_i] = U1[a]
        p64x = np.zeros((64, W64), bf)
        p64x[:, 0:72] = F2.T.astype(bf)
        p64x[:, 72:72 + D] = l1.T.astype(bf)
        p64x[:, 72 + D:72 + 2 * D] = l2.T.astype(bf)
        p64x[:, 72 + 2 * D:72 + 2 * D + SPAD] = U1SEL.astype(bf)
        p64x[:, 72 + 2 * D + SPAD:W64] = U2.T.astype(bf)
        in_maps.append({"PK72": pk72, "P64X": p64x})
        slot_maps.append(slots)

    # host assembly maps: value columns + flat offsets within a block
    col_idx = np.concatenate([np.arange(72),
                              np.repeat(72 + np.arange(NU), 2)])
    offs = [b * (N * N + 1) for b in range(72)]
    for (b, d) in plist:
        offs.append(b * N * N + d)
        offs.append(d * N * N + b)
    offs_all = np.array(offs, np.int64)
    return in_maps, slot_maps, col_idx, offs_all, SPAD, NUPAD


_CACHE = {}


def kernel(**inputs):
    from concourse.bass_utils import run_bass_kernel_spmd

    in_maps, slot_maps, col_idx, offs_all, SPAD, NUPAD = _prepare(inputs)
    key = (SPAD, NUPAD)
    nc = _CACHE.get(key)
    if nc is None:
        nc = _build_nc(SPAD, NUPAD)
        _CACHE[key] = nc
    res = run_bass_kernel_spmd(nc, in_maps, list(range(NC)))
    M = np.zeros((N * N, N * N), np.float32)
    for c in range(NC):
        out = res.results[c]["OUT"]
        slots = slot_maps[c]
        bases = np.array([a * (N * N * N) + cc * N for a, cc in slots],
                         np.int64)
        M.flat[bases[:, None] + offs_all[None, :]] = \
            out[:len(slots)][:, col_idx]
    return M


# revision 18
# speedup vs baseline: 1.0053x; 1.0053x over previous
"""Trainium2 Bass kernel for nn_Affinity (gnn_message_passing).

M[(a,b),(c,d)] = sum_{j,i} H2[a,j]H2[c,j] H1[b,i]H1[d,i] W[j,i] + diag(Mp).

Structure exploited:
 - Nonzero blocks (a,c) of M: a==c or (a,c) an edge of graph 2 -> "slots".
   626 slots total, balanced 9 bands/core across 8 cores (<=79 slots/core).
 - Within a block, only graph-1-adjacent (b,d) positions (and the diagonal)
   are nonzero; blocks are symmetric in (b,d), so each block is fully
   described by 72 diagonal values + one value per unique adjacent pair
   (275 of them) -> device output is [80 slots, 352] per core.
 - Per-slot weights factor through ZS = Xsum^T SELT (the edge-affinity
   matrix Me is never materialized), and the block values are
   OUTD = ZS^T (vv H1^T), OUTO = ZS^T (vv R) where R merges multi-edges
   of graph 1 into unique pairs. diag(Mp) folds in as an extra PSUM
   accumulation against an identity table.

All index-derived tables (incidence, SELT, R, H1T, OHSS, I72) are
host-built 0/1 matrices; every floating-point op runs on device. Host
assembly only places computed values (and zeros) into the [5184, 5184]
output.
"""
import sys
sys.path.insert(0, '/opt/trn_rl_repo')
import numpy as np

N = 72
E = 288
D = 64
NC = 8


def _split_waits(nc, limit=1):
    """This walrus build rejects instructions with >limit sem waits; move the
    excess onto same-engine NoOps inserted immediately before (same bb order =
    same engine program order, so semantics are preserved)."""
    import concourse.mybir as mybir
    for f in nc.m.functions:
        for bb in f.blocks:
            new_insts = []
            for inst in bb.instructions:
                si = inst.sync_info
                waits = list(si.on_wait) if si and si.on_wait else []
                if len(waits) > limit:
                    extra, keep = waits[:-limit], waits[-limit:]
                    for i in range(0, len(extra), limit):
                        nop = mybir.InstNoOp(
                            name=nc.get_next_instruction_name(),
                            engine=inst.engine, ins=[], outs=[],
                            sync_info=mybir.SyncInfo(
                                on_wait=extra[i:i + limit], on_update=[]),
                        )
                        nc.register_instruction(nop)
                        new_insts.append(nop)
                    si.on_wait = keep
                new_insts.append(inst)
            bb.instructions[:] = new_insts


def _incidence(src, dst):
    H = np.zeros((N, E), np.float32)
    H[src, np.arange(E)] = 1.0
    H[dst, np.arange(E)] = 1.0
    return H


def _neighbors(src, dst):
    nbrs = [set() for _ in range(N)]
    for s, d in zip(src, dst):
        nbrs[int(s)].add(int(d))
        nbrs[int(d)].add(int(s))
    return nbrs


def _plan_assignment(nbrs2, spad):
    """9 bands per core, greedily balancing slot count (1 + deg per band)."""
    deg = [len(x) for x in nbrs2]
    order = sorted(range(N), key=lambda a: -deg[a])
    cores = [[] for _ in range(NC)]
    loads = [0] * NC
    for a in order:
        c = min((c for c in range(NC) if len(cores[c]) < 9),
                key=lambda c: loads[c])
        cores[c].append(a)
        loads[c] += 1 + deg[a]
    assert max(loads) <= spad
    return cores


def _build_nc(SPAD, NUPAD):
    import concourse.bass as bass
    import concourse.mybir as mybir
    import concourse.tile as tile

    F32 = mybir.dt.float32
    BF16 = mybir.dt.bfloat16
    OUTW = 72 + NUPAD
    CW = NUPAD + 72      # combo width [s2r|s2h] / [d2r|d2h]

    nc = bass.Bass()
    # pk72: f1 [s2r|s2h] [d2r|d2h] hs ; p64x: f2t l1t l2t u1sel u2t
    W72 = D + 2 * CW + SPAD
    W64 = 72 + 2 * D + SPAD + 72
    pk72_d = nc.declare_dram_parameter("PK72", [72, W72], BF16, isOutput=False)
    p64x_d = nc.declare_dram_parameter("P64X", [64, W64], BF16, isOutput=False)
    out_d = nc.declare_dram_parameter("OUT", [SPAD, OUTW], F32, isOutput=True)

    with tile.TileContext(nc) as tc:
        with tc.tile_pool(name="cst", bufs=1) as cst, \
             tc.tile_pool(name="ps", bufs=4, space="PSUM") as ps, \
             tc.tile_pool(name="psb", bufs=4, space="PSUM") as psb:

            pk72 = cst.tile([72, W72], BF16)
            p64x = cst.tile([64, W64], BF16)
            nc.scalar.dma_start(out=p64x[:], in_=p64x_d[:])
            nc.sync.dma_start(out=pk72[:], in_=pk72_d[:])

            f1 = pk72[:, 0:D]
            sr = pk72[:, D:D + CW]                   # [s2r|s2h]
            dr = pk72[:, D + CW:D + 2 * CW]          # [d2r|d2h]
            hs = pk72[:, D + 2 * CW:D + 2 * CW + SPAD]
            f2t = p64x[:, 0:72]
            l1t = p64x[:, 72:72 + D]
            l2t = p64x[:, 72 + D:72 + 2 * D]
            u1sel = p64x[:, 72 + 2 * D:72 + 2 * D + SPAD]
            u2t = p64x[:, 72 + 2 * D + SPAD:W64]

            # relu(lamda^T) (DVE, straight from the small pack)
            r1t = cst.tile([D, D], BF16, tag="r1t")
            nc.vector.tensor_relu(out=r1t[:], in_=l1t)
            r2t = cst.tile([D, D], BF16, tag="r2t")
            nc.vector.tensor_relu(out=r2t[:], in_=l2t)

            # --- PE wave 1 (small-pack gated): AFT = F2 relu(L1)^T etc.
            aft_p = ps.tile([72, D], F32, tag="mm")
            nc.tensor.matmul(out=aft_p[:], lhsT=f2t, rhs=r1t[:],
                             start=True, stop=True)
            bft_p = ps.tile([72, D], F32, tag="mm")
            nc.tensor.matmul(out=bft_p[:], lhsT=f2t, rhs=r2t[:],
                             start=True, stop=True)
            zs_p = ps.tile([D, SPAD], F32, tag="mm")
            nc.tensor.matmul(out=zs_p[:], lhsT=f1, rhs=hs, start=True, stop=True)

            aftc = cst.tile([72, D], BF16, tag="aftc")
            nc.scalar.copy(out=aftc[:], in_=aft_p[:])
            bftc = cst.tile([72, D], BF16, tag="bftc")
            nc.vector.tensor_copy(out=bftc[:], in_=bft_p[:])
            zsc = cst.tile([D, SPAD], BF16, tag="zsc")
            nc.vector.tensor_copy(out=zsc[:], in_=zs_p[:])

            # --- PE wave 2: lamda-applied tables straight from the pack.
            vvk_p = psb.tile([D, CW], F32, tag="bb")
            nc.tensor.matmul(out=vvk_p[:], lhsT=aftc[:], rhs=sr,
                             start=True, stop=False)
            nc.tensor.matmul(out=vvk_p[:], lhsT=bftc[:], rhs=dr,
                             start=False, stop=True)

            vvkcB = cst.tile([D, NUPAD], BF16, tag="vvkcB")
            nc.vector.tensor_copy(out=vvkcB[:], in_=vvk_p[:, 0:NUPAD])
            vvkcA = cst.tile([D, 72], BF16, tag="vvkcA")
            nc.scalar.copy(out=vvkcA[:], in_=vvk_p[:, NUPAD:CW])

            # --- PE wave 3: OUT[s, 72:] = ZS^T vvm; OUT[s, :72] += Mp diag.
            bpB = psb.tile([SPAD, NUPAD], F32, tag="bb")
            nc.tensor.matmul(out=bpB[:], lhsT=zsc[:], rhs=vvkcB[:],
                             start=True, stop=True)
            bpA = psb.tile([SPAD, 72], F32, tag="bb")
            nc.tensor.matmul(out=bpA[:], lhsT=zsc[:], rhs=vvkcA[:],
                             start=True, stop=False)
            # diag(Mp) add: bpA[s, b] += sum_d U1[a_s, d] U2[b, d]
            nc.tensor.matmul(out=bpA[:], lhsT=u1sel, rhs=u2t,
                             start=False, stop=True)

            stg = cst.tile([SPAD, OUTW], F32)
            nc.scalar.copy(out=stg[:, 72:OUTW], in_=bpB[:])
            nc.vector.tensor_copy(out=stg[:, 0:72], in_=bpA[:])
            nc.sync.dma_start(out=out_d[:], in_=stg[:])

    _split_waits(nc)
    return nc


def _prepare(inputs):
    import ml_dtypes
    ins = {k: np.asarray(v) for k, v in inputs.items()}
    F1 = ins["F1"].astype(np.float32)
    F2 = ins["F2"].astype(np.float32)
    U1 = ins["U1"].astype(np.float32)
    U2 = ins["U2"].astype(np.float32)
    l1 = ins["lamda1"].astype(np.float32)
    l2 = ins["lamda2"].astype(np.float32)
    src1 = ins["src1"].astype(np.int64)
    dst1 = ins["dst1"].astype(np.int64)
    src2 = ins["src2"].astype(np.int64)
    dst2 = ins["dst2"].astype(np.int64)

    H1 = _incidence(src1, dst1)
    H2 = _incidence(src2, dst2)
    S2 = np.zeros((N, E), np.float32)
    S2[src2, np.arange(E)] = 1.0
    D2M = np.zeros((N, E), np.float32)
    D2M[dst2, np.arange(E)] = 1.0

    nbrs2 = _neighbors(src2, dst2)
    # unique unordered adjacent pairs of graph 1 + multi-edge merge R
    pairs = {}
    for i, (s, d) in enumerate(zip(src1, dst1)):
        pairs.setdefault((int(s), int(d)), []).append(i)
    plist = sorted(pairs)
    NU = len(plist)
    NUPAD = (NU + 7) // 8 * 8
    deg2 = [1 + len(x) for x in nbrs2]
    SPAD = max(80, (max(deg2) + sum(deg2) // NC + 7) // 8 * 8)
    cores = _plan_assignment(nbrs2, SPAD)

    R = np.zeros((E, NUPAD), np.float32)
    for u, key in enumerate(plist):
        for i in pairs[key]:
            R[i, u] = 1.0

    bf = ml_dtypes.bfloat16
    # host-precontracted integer tables (exact in bf16)
    S2R = S2 @ R
    D2R = D2M @ R
    S2H = S2 @ H1.T
    D2H = D2M @ H1.T

    CW = NUPAD + 72
    W72 = D + 2 * CW + SPAD
    W64 = 72 + 2 * D + SPAD + 72
    PK72B = np.zeros((72, W72), bf)
    PK72B[:, 0:D] = F1.astype(bf)
    PK72B[:, D:D + NUPAD] = S2R.astype(bf)
    PK72B[:, D + NUPAD:D + CW] = S2H.astype(bf)
    PK72B[:, D + CW:D + CW + NUPAD] = D2R.astype(bf)
    PK72B[:, D + CW + NUPAD:D + 2 * CW] = D2H.astype(bf)

    in_maps = []
    slot_maps = []
    for c in range(NC):
        slots = []
        for a in cores[c]:
            slots.append((a, a))
            for cc in sorted(nbrs2[a]):
                slots.append((a, cc))
        SELT = np.zeros((E, SPAD), np.float32)
        for s_i, (a, cc) in enumerate(slots):
            SELT[:, s_i] = H2[a] * H2[cc]
        pk72 = PK72B.copy()
        pk72[:, D + 2 * CW:D + 2 * CW + SPAD] = (H1 @ SELT).astype(bf)
        # per-core small pack with U1SEL[d, s] = U1[a_s, d] on diag slots
        U1SEL = np.zeros((64, SPAD), np.float32)
        for s_i, (a, cc) in enumerate(slots):
            if a == cc:
                U1SEL[:, s_i] = U1[a]
        p64x = np.zeros((64, W64), bf)
        p64x[:, 0:72] = F2.T.astype(bf)
        p64x[:, 72:72 + D] = l1.T.astype(bf)
        p64x[:, 72 + D:72 + 2 * D] = l2.T.astype(bf)
        p64x[:, 72 + 2 * D:72 + 2 * D + SPAD] = U1SEL.astype(bf)
        p64x[:, 72 + 2 * D + SPAD:W64] = U2.T.astype(bf)
        in_maps.append({"PK72": pk72, "P64X": p64x})
        slot_maps.append(slots)

    # host assembly maps: value columns + flat offsets within a block
    col_idx = np.concatenate([np.arange(72),
                              np.repeat(72 + np.arange(NU), 2)])
    offs = [b * (N * N + 1) for b in range(72)]
    for (b, d) in plist:
        offs.append(b * N * N + d)
        offs.append(d * N * N + b)
    offs_all = np.array(offs, np.int64)
    return in_maps, slot_maps, col_idx, offs_all, SPAD, NUPAD


_CACHE = {}


def kernel(**inputs):
    from concourse.bass_utils import run_bass_kernel_spmd

    in_maps, slot_maps, col_idx, offs_all, SPAD, NUPAD = _prepare(inputs)
    key = (SPAD, NUPAD)
    nc = _CACHE.get(key)
    if nc is None:
        nc = _build_nc(SPAD, NUPAD)
        _CACHE[key] = nc
    res = run_bass_kernel_spmd(nc, in_maps, list(range(NC)))
    M = np.zeros((N * N, N * N), np.float32)
    for c in range(NC):
        out = res.results[c]["OUT"]
        slots = slot_maps[c]
        bases = np.array([a * (N * N * N) + cc * N for a, cc in slots],
                         np.int64)
        M.flat[bases[:, None] + offs_all[None, :]] = \
            out[:len(slots)][:, col_idx]
    return M
